# revision 71
# baseline (speedup 1.0000x reference)
"""Trainium2 Bass kernel for nn_AutoEncoder_64854006170336.

Per-joint-embedding transformer encoder (B=1024, A=25 tokens, D=512, H=8, L=6).
Data-parallel over batch: 8 cores x 128 batches each. No collectives.

Design (vs the original pack-contiguous baseline):
- Embedding computed directly in D-major (stationary We chunk, moving x batch
  cols) with strided DVE evacs into hT; no DRAM round-trip.
- Attention runs on 100-token packs (4 batches), but the o-projection /
  residual / LayerNorm phase is retiled to 25 full-width 128-token windows of
  the contiguous oTall / hT buffers (all consumers use free-dim windows, so
  the granularities may differ): full PE stationary width, 22% fewer LN
  chains, and a flat-row output store.
- Residual + prev-layer LN gain folded onto the PE: each o-tile psum
  accumulates bias (ones-row x bo), hT_win^T @ diag(g_{l-1}) (host-shipped
  diagonal blocks), then oT^T @ Wo; the psum is evacuated once to SBUF (ACT)
  so the LN chain reads SBUF and the psum slot recycles ~2us earlier.
- Next layer's kproj tiles are emitted inside the attention loop as soon as
  the hT columns they read have been flushed, filling PE bubbles in the
  engine-balanced attention phase instead of running as a serial block.
- AV softmax denominator merged into the AV matmul: v tiles carry a ones
  column ([PT, H, 65]), two psum tiles [PT, 4, 65], reciprocal reads col 64,
  normalization folded into the psum evacuation.
- Wq shipped and computed in fp8e4m3 (q-projection only; scores scale ~0.2 so
  the quantization shifts attention weights <1%); the moving operand is hT
  directly -- the PE accepts fp8-stationary x bf16-moving mixed matmuls.
- LN rstd = rsqrt(var+eps) computed on DVE (bit-trick seed + 2 Newton
  iterations, ~5e-6 rel).  Keeping Sqrt off the ACT engine matters: Sqrt and
  Exp live in different activation-table sets and the scheduler interleaves
  softmax Exp evacs with the LN chains, so an ACT Sqrt forced a 1.3us
  activation-table reload per pack (242 reloads per exec).
- Weight DMAs batched (Wk|Wv|Wo one tensor/layer, bq per layer, bk/bo all
  layers once) on the gpsimd SWDGE queue, prefetched one layer ahead; Wq
  streamed 7 tiles deep on the scalar HWDGE ring.
- Token-major -> D-major transposes (attention output, LN output) run on the
  tensor engine (is_transpose matmuls against a [128,128] identity) instead of
  xbar DMA transposes: the single in-order HWDGE ring serialized 64 transposes
  per layer and left every engine waiting on it.  PE transposes also lift the
  mult-of-16 pad requirement, so packs are stored unpadded (PTP == PT == 100).
"""

from contextlib import ExitStack

import os

import numpy as np
import ml_dtypes

import concourse.bass as bass
import concourse.mybir as mybir
import concourse.tile as tile
from concourse import bacc
from concourse.bass_utils import run_bass_kernel_spmd

BF = ml_dtypes.bfloat16
bf16 = mybir.dt.bfloat16
f8 = mybir.dt.float8e4
F8 = ml_dtypes.float8_e4m3
f32 = mybir.dt.float32
u32 = mybir.dt.uint32
AF = mybir.ActivationFunctionType
ALU = mybir.AluOpType
RSQRT_MAGIC = 0x5F3759DF

B, J, DI, D, H, L = 1024, 24, 64, 512, 8, 6
A = J + 1            # 25 tokens
NCORES = 8
BC = B // NCORES     # 128 batches/core
G = 4                # batches per pack
NPACK = BC // G      # 32
PT = G * A           # 100 live tokens per pack
PTP = PT             # pack stride (unpadded; PE transposes need no alignment)
TW = NPACK * PTP     # 3200 D-major token columns
DEPTH = D // H       # 64
NCH = 4
LN_EPS = 1e-5
CAT = 4              # packs per score-psum concat
TOK_TILES = [(i * 512, 512) for i in range(6)] + [(3072, 128)]
CATS = [(g0, min(CAT, NPACK - g0)) for g0 in range(0, NPACK, CAT)]
JB = 3               # joints per embedding weight block

_compiled = None


def _ap(tensor_ap, extra_offset, dims):
    return bass.AP(tensor=tensor_ap.tensor, offset=tensor_ap.offset + extra_offset,
                   ap=dims)


def _v(ap, dims, extra=0):
    """Replace free dims of an AP (keep partition dim), offset in elements."""
    return bass.AP(tensor=ap.tensor, offset=ap.offset + extra,
                   ap=[ap.ap[0], *dims])


class _Kern:
    def __init__(self):
        self.nc = bacc.Bacc(None, target_bir_lowering=False)
        nc = self.nc
        self.x_t = nc.dram_tensor("x_t", [DI, J, BC], bf16, kind="ExternalInput")
        self.WeT_t = nc.dram_tensor("WeT_t", [DI, J, NCH, 128], bf16,
                                    kind="ExternalInput")
        self.beR_t = nc.dram_tensor("beR_t", [128, J, NCH], f32,
                                    kind="ExternalInput")
        self.clsR_t = nc.dram_tensor("clsR_t", [128, NCH], bf16,
                                     kind="ExternalInput")
        self.WKVO_t = nc.dram_tensor("WKVO_t", [L, 128, 3, NCH, D], bf16,
                                     kind="ExternalInput")
        self.Wq_t = nc.dram_tensor("Wq_t", [L, A, 128, NCH, D], f8,
                                   kind="ExternalInput")
        self.Dg_t = nc.dram_tensor("Dg_t", [L, 128, NCH, 128], bf16,
                                   kind="ExternalInput")
        self.bkAll_t = nc.dram_tensor("bkAll_t", [128, L, NCH], f32,
                                      kind="ExternalInput")
        self.bqL_t = nc.dram_tensor("bqL_t", [L, 128, A, NCH], f32,
                                    kind="ExternalInput")
        self.boAll_t = nc.dram_tensor("boAll_t", [1, L, D], bf16,
                                      kind="ExternalInput")
        self.g5_row = nc.dram_tensor("g5_row", [1, D], f32, kind="ExternalInput")
        self.b5_row = nc.dram_tensor("b5_row", [1, D], f32, kind="ExternalInput")
        self.MU_t = nc.dram_tensor("MU_t", [G + 1, PT], bf16, kind="ExternalInput")
        self.MV_t = nc.dram_tensor("MV_t", [G + 1, CAT * PT], bf16,
                                   kind="ExternalInput")
        self.I128_t = nc.dram_tensor("I128_t", [128, 128], bf16,
                                     kind="ExternalInput")
        self.out_t = nc.dram_tensor("out", [BC, A, D], f32, kind="ExternalOutput")

    def build(self):
        nc = self.nc
        with ExitStack() as ctx:
            tc = ctx.enter_context(tile.TileContext(nc))
            self.tc = tc
            p = lambda name, bufs, space="SBUF": ctx.enter_context(
                tc.tile_pool(name=name, bufs=bufs, space=space)
            )
            self.big = p("big", 1)
            self.wkvop = p("wkvop", 2)
            self.wqp = p("wqp", 7)
            self.vstore = p("vstore", 10)
            self.xtp = p("xtp", 12)
            self.stage = p("stage", 3)
            self.ypool = p("ypool", 3)
            self.otokp = p("otok", 5)
            self.fpool = p("fpool", 2)
            self.stats = p("stats", 8)
            self.psA = p("psA", 2, "PSUM")
            self.psS = p("psS", 2, "PSUM")
            self.psV = p("psV", 2, "PSUM")
            self.psT = p("psT", 2, "PSUM")
            self._consts()
            self._pend_xh = None
            self.next_w = self._load_layer_weights(0)
            self._embedding()
            self.wq_tiles = self._emit_wq_loads(0)
            for l in range(L):
                self._layer(l)
        nc.compile()
        return nc

    def _consts(self):
        nc, big = self.nc, self.big
        self.hT = big.tile([128, NCH, TW], bf16, tag="hT")
        self.kT = big.tile([128, NCH, TW], bf16, tag="kT")
        self.qT = big.tile([128, NCH, TW], bf16, tag="qT")
        self.oTall = big.tile([128, NCH, TW], bf16, tag="oTall")
        self.MU = big.tile([G + 1, PT], bf16, tag="MU")
        self.MV = big.tile([G + 1, CAT * PT], bf16, tag="MV")
        self.I128 = big.tile([128, 128], bf16, tag="I128")
        self.ones_row = big.tile([1, 128], bf16, tag="ones_row")
        self.eps_t = big.tile([128, 1], f32, tag="eps")
        self.g5b = big.tile([128, D], f32, tag="g5b")
        self.b5b = big.tile([128, D], f32, tag="b5b")
        self.bkAll = big.tile([128, L, NCH], f32, tag="bkAll")
        self.boAll = big.tile([1, L, D], bf16, tag="boAll")
        self.beR = big.tile([128, J, NCH], f32, tag="beR")
        self.clsR = big.tile([128, NCH], bf16, tag="clsR")
        nc.gpsimd.dma_start(out=self.MU[:], in_=self.MU_t[:])
        nc.gpsimd.dma_start(out=self.MV[:], in_=self.MV_t[:])
        nc.gpsimd.dma_start(out=self.I128[:], in_=self.I128_t[:])
        nc.gpsimd.dma_start(out=self.bkAll[:], in_=self.bkAll_t[:])
        nc.gpsimd.dma_start(out=self.boAll[:], in_=self.boAll_t[:])
        nc.gpsimd.dma_start(out=self.beR[:], in_=self.beR_t[:])
        nc.gpsimd.dma_start(out=self.clsR[:], in_=self.clsR_t[:])
        nc.vector.memset(self.ones_row[:], 1.0)
        nc.vector.memset(self.eps_t[:], LN_EPS)
        self.magic = big.tile([128, 1], u32, tag="magic")
        nc.vector.memset(self.magic[:], RSQRT_MAGIC)
        nc.gpsimd.dma_start(out=self.g5b[:], in_=self.g5_row[:].to_broadcast((128, D)))
        nc.gpsimd.dma_start(out=self.b5b[:], in_=self.b5_row[:].to_broadcast((128, D)))

    def _load_layer_weights(self, l):
        nc = self.nc
        wkvo = self.wkvop.tile([128, 3, NCH, D], bf16, tag="wkvo",
                               name=f"wkvo{l}")
        nc.gpsimd.dma_start(out=wkvo[:], in_=self.WKVO_t[l])
        dg = self.wkvop.tile([128, NCH, 128], bf16, tag="dg", name=f"dg{l}")
        nc.gpsimd.dma_start(out=dg[:], in_=self.Dg_t[l])
        bq = self.wkvop.tile([128, A, NCH], f32, tag="bq", name=f"bq{l}")
        nc.gpsimd.dma_start(out=bq[:], in_=self.bqL_t[l])
        return wkvo, dg, bq

    def _pos_ap(self, big3, kc, a):
        """Moving-operand AP: [128, NPACK, G] = cols PTP*g + 25j + a of chunk kc."""
        base = big3[:, kc, :]
        return _ap(base, a, [base.ap[0], [PTP, NPACK], [A, G]])

    def _pos_out_ap(self, big3, a):
        """[128, NCH, NPACK, G] strided evac target across all chunks."""
        base = big3[:]
        return _ap(base, a, [base.ap[0], [TW, NCH], [PTP, NPACK], [A, G]])

    def _head_win(self, tens, h, g):
        return tens[(h % 2) * 64 : (h % 2) * 64 + 64, h // 2,
                    g * PTP : g * PTP + PT]

    def _embedding(self):
        nc = self.nc
        with self.tc.tile_pool(name="embp", bufs=2) as embp:
            xall = embp.tile([DI, J, BC], bf16, tag="xall", bufs=1)
            nc.gpsimd.dma_start(out=xall[:], in_=self.x_t[:])
            for jb in range(J // JB):
                web = embp.tile([DI, JB, NCH, 128], bf16, tag="web")
                nc.scalar.dma_start(
                    out=web[:], in_=self.WeT_t[:, jb * JB : (jb + 1) * JB, :, :]
                )
                for ji in range(JB):
                    j = jb * JB + ji
                    ps = self.psA.tile([128, NCH, 128], f32, tag="pp",
                                       name=f"eps{j}")
                    for oc in range(NCH):
                        nc.tensor.matmul(
                            ps[:, oc, :], web[:, ji, oc, :], xall[:, j, :],
                            start=True, stop=True,
                        )
                    nc.vector.tensor_tensor(
                        self._pos_out_ap(self.hT, j + 1),
                        _v(ps[:], [[128, NCH], [G, NPACK], [1, G]]),
                        _v(self.beR[:], [[1, NCH], [0, NPACK], [0, G]],
                           extra=j * NCH),
                        ALU.add,
                    )
        nc.vector.tensor_copy(
            self._pos_out_ap(self.hT, 0),
            _v(self.clsR[:], [[1, NCH], [0, NPACK], [0, G]]),
        )

    def _kproj_tile(self, l, wkvo, c0, cn):
        nc = self.nc
        for oc in range(NCH):
            ps = self.psA.tile([128, D], f32, tag="pp", name="kps")
            for kc in range(NCH):
                nc.tensor.matmul(
                    ps[:, :cn],
                    wkvo[:, 0, kc, oc * 128 : (oc + 1) * 128],
                    self.hT[:, kc, c0 : c0 + cn],
                    start=(kc == 0),
                    stop=(kc == NCH - 1),
                )
            nc.scalar.activation(
                self.kT[:, oc, c0 : c0 + cn], ps[:, :cn],
                AF.Identity, bias=self.bkAll[:, l, oc : oc + 1], scale=1.0,
            )

    def _kproj(self, l, wkvo):
        for (c0, cn) in TOK_TILES:
            self._kproj_tile(l, wkvo, c0, cn)

    def _qproj(self, l, bq):
        nc = self.nc
        for a in range(A):
            wq = self.wq_tiles[a]
            ps = self.psA.tile([128, NCH, 128], f32, tag="pp", name="qps")
            for oc in range(NCH):
                for kc in range(NCH):
                    nc.tensor.matmul(
                        ps[:, oc, :],
                        wq[:, kc, oc * 128 : (oc + 1) * 128],
                        self._pos_ap(self.hT, kc, a),
                        start=(kc == 0),
                        stop=(kc == NCH - 1),
                    )
            nc.vector.tensor_tensor(
                self._pos_out_ap(self.qT, a),
                _v(ps[:], [[128, NCH], [G, NPACK], [1, G]]),
                _v(bq[:], [[1, NCH], [0, NPACK], [0, G]], extra=a * NCH),
                ALU.add,
            )

    def _vproj(self, wkvo, v_sb, packs):
        """v projection for the given packs (called lazily per score group so
        the evacs never sit queued ahead of attention's per-pack chains).

        v tiles live in a fixed ring: the ones-column (softmax denominator)
        is written once per slot at startup and survives slot reuse, and the
        psum evac runs on ACT to keep the DVE FIFO short."""
        nc = self.nc
        for g in packs:
            ps = self.psA.tile([128, D], f32, tag="pp", name="vps")
            for kc in range(NCH):
                nc.tensor.matmul(
                    ps[:PT, :],
                    self.hT[:, kc, g * PTP : g * PTP + PT],
                    wkvo[:, 1, kc, :],
                    start=(kc == 0),
                    stop=(kc == NCH - 1),
                )
            vt = self.vstore.tile([PT, H, 65], bf16, tag="v")
            nc.vector.tensor_copy(
                _v(vt[:], [[65, H], [1, DEPTH]]),
                _v(ps[:PT, :], [[DEPTH, H], [1, DEPTH]]),
            )
            nc.vector.memset(vt[:, :, 64:65], 1.0)
            v_sb[g] = vt

    def _attention(self, l, wkvo, dg):
        nc = self.nc
        v_sb = [None] * NPACK
        for gi, (g0, ng) in enumerate(CATS):
            self._vproj(wkvo, v_sb, range(g0, g0 + ng))
            xts = []
            for hp in range(H // 2):
                sc2 = [self.psS.tile([PT, CAT * PT], f32, tag="sc",
                                     name=f"sc{u}") for u in range(2)]
                for u in range(2):
                    nc.tensor.matmul(
                        sc2[u][:, : ng * PT], self.MU[:], self.MV[:, : ng * PT],
                        start=True, stop=False,
                    )
                for i in range(ng):
                    for u in range(2):
                        h = 2 * hp + u
                        nc.tensor.matmul(
                            sc2[u][:, i * PT : (i + 1) * PT],
                            self._head_win(self.kT, h, g0 + i),
                            self._head_win(self.qT, h, g0 + i),
                            start=False, stop=(i == ng - 1),
                        )
                for u in range(2):
                    xt = self.xtp.tile([PT, CAT * PT], bf16, tag="xt")
                    nc.scalar.activation(
                        xt[:, : ng * PT], sc2[u][:, : ng * PT], AF.Exp,
                        scale=1.0 / 8.0,
                    )
                    xts.append(xt)
            for i in range(ng):
                g = g0 + i
                self._av(xts, v_sb, g0, i)
                # o-projection tiles (128 tokens) as soon as their oTall
                # columns are complete
                while (self._op_next < TW // 128
                       and 128 * (self._op_next + 1) <= PTP * (g + 1)):
                    self._opack(l, wkvo, dg, self._op_next)
                    self._op_next += 1
                # next layer's kproj, tile by tile as soon as the hT columns
                # it reads have been flushed (tile t's opack flushes t-1):
                # fills PE bubbles in the engine-balanced attention phase and
                # shrinks the serial layer-start
                if l < L - 1:
                    while self._kp_next < len(TOK_TILES) - 1:
                        c0, cn = TOK_TILES[self._kp_next]
                        if 128 * (self._op_next - 1) < c0 + cn:
                            break
                        self._kproj_tile(l + 1, self.next_w[0], c0, cn)
                        self._kp_next += 1
            if gi == 0 and l < L - 1:
                self.next_w = self._load_layer_weights(l + 1)
                self.wq_tiles = self._emit_wq_loads(l + 1)
        self._flush_hT()
        if l < L - 1:
            c0, cn = TOK_TILES[-1]
            self._kproj_tile(l + 1, self.next_w[0], c0, cn)

    def _av(self, xts, v_sb, g0, i):
        nc = self.nc
        g = g0 + i
        ava = self.psV.tile([PT, 4, 65], f32, tag="ava", bufs=1)
        avb = self.psV.tile([PT, 4, 65], f32, tag="avb", bufs=1)
        for h in range(H):
            ps = ava if h < 4 else avb
            xsl = xts[h][:, i * PT : (i + 1) * PT]
            nc.tensor.matmul(
                ps[:, h % 4, :], xsl, v_sb[g][:, h, :], start=True, stop=True
            )
        reca = self.stats.tile([PT, 4, 1], f32, tag="reca")
        nc.vector.reciprocal(reca[:], ava[:, :, 64:65])
        recb = self.stats.tile([PT, 4, 1], f32, tag="recb")
        nc.vector.reciprocal(recb[:], avb[:, :, 64:65])
        ot = self.otokp.tile([PT, D], bf16, tag="otok")
        nc.vector.tensor_tensor(
            _v(ot[:, :], [[DEPTH, 4], [1, DEPTH]]),
            ava[:, :, 0:64],
            _v(reca[:], [[1, 4], [0, DEPTH]]),
            ALU.mult,
        )
        nc.vector.tensor_tensor(
            _v(ot[:, :], [[DEPTH, 4], [1, DEPTH]], extra=256),
            avb[:, :, 0:64],
            _v(recb[:], [[1, 4], [0, DEPTH]]),
            ALU.mult,
        )
        pst = self.psT.tile([128, NCH, 128], bf16, tag="tp", name="oTt")
        for c in range(NCH):
            nc.tensor.matmul(
                pst[:, c, :PT], ot[:, c * 128 : (c + 1) * 128],
                self.I128[:PT, :PT], start=True, stop=True, is_transpose=True,
            )
        nc.vector.tensor_copy(
            self.oTall[:, :, g * PTP : (g + 1) * PTP],
            pst[:, :, :PT],
        )

    def _flush_hT(self):
        """Emit the delayed hT transpose for the previous tile's LN output.

        Delayed one tile so the PE has a tile's worth of matmuls between the
        LN chain that produces xh and the transpose that consumes it."""
        if self._pend_xh is None:
            return
        t, xh = self._pend_xh
        self._pend_xh = None
        nc = self.nc
        c0 = 128 * t
        pst = self.psT.tile([128, NCH, 128], bf16, tag="tp", name="hTt")
        for c in range(NCH):
            nc.tensor.matmul(
                pst[:, c, :], xh[:, c * 128 : (c + 1) * 128], self.I128[:],
                start=True, stop=True, is_transpose=True,
            )
        nc.scalar.copy(self.hT[:, :, c0 : c0 + 128], pst[:])

    def _opack(self, l, wkvo, dg, t):
        """o-projection + residual + LN for token tile t (128 tokens).

        Attention runs on 100-token packs, but the o-projection/LN phase is
        retiled to 25 full-width 128-token windows of the contiguous oTall /
        hT buffers: full PE stationary width and 22% fewer LN chains."""
        nc = self.nc
        c0 = 128 * t
        ps = self.psA.tile([128, D], f32, tag="pp", name="ops")
        # bias first: full-width start=True whose deps (consts) are ready
        # immediately, so the scheduler cannot hoist an accumulate above it
        nc.tensor.matmul(
            ps[:, :], self.ones_row[:, :], self.boAll[:, l, :],
            start=True, stop=False,
        )
        # residual + prev-layer LN gain: ps += hT_win^T @ diag(g_{l-1})
        for kc in range(NCH):
            nc.tensor.matmul(
                ps[:, kc * 128 : (kc + 1) * 128],
                self.hT[:, kc, c0 : c0 + 128],
                dg[:, kc, :],
                start=False, stop=False,
            )
        # o-projection last (waits on the oT evacs, so it finishes last)
        for kc in range(NCH):
            nc.tensor.matmul(
                ps[:, :],
                self.oTall[:, kc, c0 : c0 + 128],
                wkvo[:, 2, kc, :],
                start=False, stop=(kc == NCH - 1),
            )
        # evacuate the psum once (ACT) and run the whole LN chain off the
        # SBUF copy: the psum slot frees ~2us earlier, unblocking the next
        # tile's matmuls through the 2-deep psA rotation
        ysb = self.ypool.tile([128, D], f32, tag="ysb")
        nc.scalar.copy(ysb[:], ps[:, :])
        self._flush_hT()
        st6 = self.stats.tile([128, 6], f32, tag="st6")
        nc.vector.bn_stats(st6[:], ysb[:, :])
        mv = self.stats.tile([128, 2], f32, tag="mv")
        nc.vector.bn_aggr(mv[:], st6[:])
        # rstd = rsqrt(var+eps) via DVE Newton (bit-trick seed, 2 iters,
        # ~5e-6 rel).  Keeping Sqrt off the ACT engine matters: Sqrt and Exp
        # live in different activation-table sets, and the scheduler
        # interleaves softmax Exp with the LN chain, so an ACT Sqrt forced a
        # 1.3us table reload per pack (242 reloads, 310us/exec).
        ve = self.stats.tile([128, 1], f32, tag="ve")
        nc.vector.tensor_scalar(ve[:], mv[:, 1:2], LN_EPS, None, ALU.add)
        hsh = self.stats.tile([128, 1], u32, tag="hsh")
        nc.vector.tensor_scalar(hsh[:], ve[:].bitcast(u32), 1, None,
                                ALU.logical_shift_right)
        y0 = self.stats.tile([128, 1], f32, tag="y0")
        nc.vector.tensor_tensor(y0[:].bitcast(u32), self.magic[:, :], hsh[:],
                                ALU.subtract)
        y1 = self.stats.tile([128, 1], f32, tag="y1")
        na = self.stats.tile([128, 1], f32, tag="na")
        nc2 = self.stats.tile([128, 1], f32, tag="nc2")
        for it in range(2):
            src, dst = (y0, y1) if it == 0 else (y1, y0)
            nc.vector.tensor_tensor(na[:], src[:], src[:], ALU.mult)
            nc.vector.tensor_tensor(na[:], na[:], ve[:], ALU.mult)
            nc.vector.tensor_scalar(nc2[:], na[:], -0.5, 1.5, ALU.mult, ALU.add)
            nc.vector.tensor_tensor(dst[:], src[:], nc2[:], ALU.mult)
        rstd = y0
        nmr = self.stats.tile([128, 1], f32, tag="nmr")
        nc.vector.tensor_scalar(
            nmr[:], mv[:, 0:1], rstd[:], -1.0, ALU.mult, ALU.mult
        )
        if l < L - 1:
            xh = self.stage.tile([128, D], bf16, tag="xhat", name="xh")
            nc.scalar.activation(
                xh[:, :], ysb[:, :], AF.Identity, bias=nmr[:], scale=rstd[:]
            )
            self._pend_xh = (t, xh)
        else:
            of = self.fpool.tile([128, D], f32, tag="of")
            nc.scalar.activation(
                of[:], ysb[:, :], AF.Identity, bias=nmr[:], scale=rstd[:]
            )
            nc.vector.tensor_mul(of[:], of[:], self.g5b[:, :])
            nc.vector.tensor_add(of[:], of[:], self.b5b[:, :])
            nc.sync.dma_start(
                out=_ap(self.out_t[:], c0 * D, [[D, 128], [1, D]]), in_=of[:]
            )

    def _emit_wq_loads(self, l):
        nc = self.nc
        tiles = []
        for a in range(A):
            wq = self.wqp.tile([128, NCH, D], f8, tag="wq", name=f"wq{l}_{a}")
            nc.scalar.dma_start(out=wq[:], in_=self.Wq_t[l, a])
            tiles.append(wq)
        return tiles

    def _layer(self, l):
        wkvo, dg, bq = self.next_w
        if l == 0:
            self._kproj(0, wkvo)
        self._qproj(l, bq)
        self._kp_next = 0
        self._op_next = 0
        self._attention(l, wkvo, dg)


def _build():
    return _Kern().build()


def _prep_inputs(inputs):
    """Host-side fold + layout prep. Returns (shared dict, per-core x list)."""
    f = lambda v: np.asarray(v, dtype=np.float64)
    x = np.asarray(inputs["x"], dtype=np.float32)
    We, be = f(inputs["We"]), f(inputs["be"])
    cls_token = f(inputs["cls_token"])
    Wk, bk = f(inputs["Wk"]), f(inputs["bk"])
    Wv, bv = f(inputs["Wv"]), f(inputs["bv"])
    Wq, bq = f(inputs["Wq"]), f(inputs["bq"])
    Wo, bo = f(inputs["Wo"]), f(inputs["bo"])
    ln_g, ln_b = f(inputs["ln_g"]), f(inputs["ln_b"])

    def chunk_w(w):  # [512, 512] -> [128, 4, 512]
        return np.ascontiguousarray(
            w.reshape(NCH, 128, D).transpose(1, 0, 2)
        ).astype(BF)

    def chunk_b(b):  # [512] -> [128, 4]
        return np.ascontiguousarray(b.reshape(NCH, 128).T).astype(np.float32)

    WKVO = np.zeros((L, 128, 3, NCH, D), BF)
    Wq_t = np.zeros((L, A, 128, NCH, D), F8)
    Dg = np.zeros((L, 128, NCH, 128), BF)
    bkAll = np.zeros((128, L, NCH), np.float32)
    bqL = np.zeros((L, 128, A, NCH), np.float32)
    boAll = np.zeros((1, L, D), BF)
    idx = np.arange(128)
    for l in range(L):
        gf = ln_g[l - 1] if l > 0 else np.ones(D)
        bf_ = ln_b[l - 1] if l > 0 else np.zeros(D)
        bv_f = bf_ @ Wv[l] + bv[l]
        WKVO[l, :, 0] = chunk_w(gf[:, None] * Wk[l])
        WKVO[l, :, 1] = chunk_w(gf[:, None] * Wv[l])
        WKVO[l, :, 2] = chunk_w(Wo[l])
        bkAll[:, l, :] = chunk_b(bf_ @ Wk[l] + bk[l])
        boAll[0, l] = (bv_f @ Wo[l] + bo[l] + bf_).astype(BF)
        gc = gf.reshape(NCH, 128)  # [c, q]
        for c in range(NCH):
            Dg[l, idx, c, idx] = gc[c].astype(BF)
        for a in range(A):
            Wq_t[l, a] = chunk_w(gf[:, None] * Wq[l, a]).astype(F8)
            bqL[l, :, a, :] = chunk_b(bf_ @ Wq[l, a] + bq[l, a])

    MB = 400.0  # exp(-400/8) == 0 in bf16; diag contributions cancel exactly
    MU = np.zeros((G + 1, PT), BF)
    MV = np.zeros((G + 1, CAT * PT), BF)
    MU[0, :] = 1.0
    MV[0, :] = -MB
    for j in range(G):
        MU[1 + j, j * A : (j + 1) * A] = 1.0
        for i in range(CAT):
            MV[1 + j, i * PT + j * A : i * PT + (j + 1) * A] = MB

    WeT = np.ascontiguousarray(We.transpose(1, 0, 2)).reshape(DI, J, NCH, 128)
    beR = np.ascontiguousarray(be.reshape(J, NCH, 128).transpose(2, 0, 1))
    clsR = np.ascontiguousarray(cls_token.reshape(NCH, 128).T)

    shared = {
        "WeT_t": WeT.astype(BF),
        "beR_t": beR.astype(np.float32),
        "clsR_t": clsR.astype(BF),
        "WKVO_t": WKVO, "Wq_t": Wq_t, "Dg_t": Dg,
        "bkAll_t": bkAll, "bqL_t": bqL, "boAll_t": boAll,
        "g5_row": ln_g[L - 1].reshape(1, D).astype(np.float32),
        "b5_row": ln_b[L - 1].reshape(1, D).astype(np.float32),
        "MU_t": MU, "MV_t": MV,
        "I128_t": np.eye(128, dtype=BF),
    }
    x_cores = []
    for c in range(NCORES):
        xc = x[c * BC : (c + 1) * BC]            # [128, 24, 64]
        x_cores.append(np.ascontiguousarray(xc.transpose(2, 1, 0)).astype(BF))
    return shared, x_cores


def kernel(**inputs) -> np.ndarray:
    global _compiled
    if _compiled is None:
        _compiled = _build()
    nc = _compiled
    shared, x_cores = _prep_inputs(inputs)
    in_maps = [{**shared, "x_t": x_cores[c]} for c in range(NCORES)]
    res = run_bass_kernel_spmd(nc, in_maps, core_ids=list(range(NCORES)))
    return np.concatenate([r["out"] for r in res.results], axis=0)

